# revision 27
# baseline (speedup 1.0000x reference)
"""Trainium2 Bass kernel for the arm-sampling rollout problem.

Math: the reference's 2048-step scan x <- x - (A@x)*dt with
A = P diag(exp(D)) P^-1 has the closed form
    hidden[k] = P diag(lam_i^k) P^-1 x0,   lam_i = 1 - dt*exp(D_i)
so actions^T[ch, k] = tanh(sum_i G[ch,i] * c_i * lam_i^k + bm[ch]) with
G = Wm @ P and c = P^-1 x0 (on-device Gauss-Jordan on [P^T | I]).
The output is the memory-bound broadcast
    out[arm, j] = 150*eps[arm, j] + 15000*act_flat[j]
over a [5000, 4096] array, 625 arms per core across 8 cores.

Key scheduling facts measured from NTFF profiles:
- A dma_start's descriptors (16KB per partition-row) are spread over
  the 16 DMA engines (26.8GB/s each) only while other transfers are
  co-resident in the queue's dispatch window; a transfer alone in the
  window crawls on ~1 engine. Each HWDGE queue also only admits ~5
  outstanding dma_starts (later triggers stall). So bulk I/O is split
  into ~1MB transfers across BOTH HWDGE queues (sync+scalar), with
  small trailing dummy transfers so no real transfer drains alone.
- PE fp32 matmul costs ~2.1us per 512 cols; bf16 ~0.55us. The action
  row is computed as ONE [8,512] fp32 matmul via the geometric-series
  split lam^(k+512j) = lam^(512j) * lam^k (lam^(512j) columns folded
  into 8 stacked lhsT columns), tanh'd to bf16, and broadcast 128-wide
  by selector-matrix bf16 matmuls (rhs = full [8,512] tile at base
  partition 0, so no cross-partition hop is needed).
- All small parameters, the identity/row-selector tables, and the
  step-index row are host-packed into three small partition-layout
  arrays (pk3, pk10a, pk10b), each loading as a handful of
  descriptors; gpsimd runs nothing but the tiny GJ column preps.
"""

import numpy as np

import concourse.bass as bass
import concourse.bacc as bacc
import concourse.mybir as mybir
import concourse.tile as tile
from concourse.bass_utils import run_bass_kernel_spmd

N_ARMS = 5000
N_STEPS = 2048
H = 10
F = 2 * N_STEPS  # 4096 flattened per-arm elements
N_CORES = 8
ARMS_PER_CORE = N_ARMS // N_CORES  # 625
FP = mybir.dt.float32
BF = mybir.dt.bfloat16

_NC_CACHE: dict = {}


def build_nc():
    AFT = mybir.ActivationFunctionType
    ALU = mybir.AluOpType

    nc = bacc.Bacc(
        "TRN2",
        target_bir_lowering=False,
        debug=False,
        enable_asserts=True,
        num_devices=N_CORES,
    )

    eps_d = nc.dram_tensor("eps", [ARMS_PER_CORE, F], FP, kind="ExternalInput")
    pk3_d = nc.dram_tensor("pk3", [3, 257], FP, kind="ExternalInput")
    pk10a_d = nc.dram_tensor("pk10a", [10, 136], FP, kind="ExternalInput")
    pk10b_d = nc.dram_tensor("pk10b", [10, 2304], FP, kind="ExternalInput")
    out_d = nc.dram_tensor("out", [ARMS_PER_CORE, F], FP, kind="ExternalOutput")
    dscr_d = nc.dram_tensor("dscr", [32, F], FP, kind="Internal")

    # DMA facts (measured): descriptors are one 16KB row each and only 16KB
    # descriptors get spread across the 16 DMA engines (bigger ones crawl on
    # one engine); each queue's descriptor ring holds ~512 and admits whole
    # transfers only; transfers triggered back-to-back co-finish at ~430GB/s
    # aggregate, stragglers crawl. So: 4 full 128-row tiles burst on sync
    # (512 descriptors exactly) and the 113-row tile rides the scalar
    # queue's initial burst alongside the packed smalls.
    TILE_ROWS = [(0, 128), (128, 256), (256, 384), (384, 512), (512, 625)]

    with tile.TileContext(nc) as tc:
        with (
            tc.tile_pool(name="sbc", bufs=1) as sbc,
            tc.tile_pool(name="sbgj", bufs=2) as sbgj,
            tc.tile_pool(name="sbeps", bufs=1) as sbeps,
            tc.tile_pool(name="psa", bufs=3, space=bass.MemorySpace.PSUM) as psa,
            tc.tile_pool(name="psbc", bufs=2, space=bass.MemorySpace.PSUM) as psbc,
            tc.tile_pool(name="psB", bufs=2, space=bass.MemorySpace.PSUM) as psB,
        ):
            # ---------- selector table built pre-input ------------------------
            # selm[:, 128r:128(r+1)] is the [8,128] lhsT that broadcasts row r
            seli = sbc.tile([8, 8 * 128], mybir.dt.int32, tag="seli")
            nc.gpsimd.iota(
                seli[:], pattern=[[-1, 8], [0, 128]], base=0, channel_multiplier=1
            )
            selm = sbc.tile([8, 8 * 128], BF, tag="selm")
            nc.vector.tensor_scalar(selm[:], seli[:], 0, None, ALU.is_equal)

            # ---------- sync HWDGE: pk3 + input groups + dummy ----------------
            # Order per the measured credit/dealing behavior: small tile-4
            # pieces ride the first tight window; t2/t3 join as credits free.
            pk3 = sbc.tile([3, 257], FP, tag="pk3")
            nc.sync.dma_start(pk3[:], pk3_d.ap())
            eps_tiles = []
            for r0, r1 in TILE_ROWS:
                t = sbeps.tile([128, F], FP, tag="eps" + str(r0))
                eps_tiles.append((t, r0, r1 - r0))
            # {pk3,t0,t1,t4a,t4b,t2} = 500 descriptors, admitted as one burst;
            # t3 enters as 4 small pieces so each joins the active stream
            # within a few us of dealing start (late deep joiners crawl)
            IN_GROUPS = [
                (0, 128), (128, 256), (512, 569), (569, 625), (256, 384),
                (384, 416), (416, 448), (448, 480), (480, 512),
            ]
            for g0, g1 in IN_GROUPS:
                ti = min(g0 // 128, 4)
                t, r0, _ = eps_tiles[ti]
                nc.sync.dma_start(t[g0 - r0 : g1 - r0, :], eps_d.ap()[g0:g1, :])
            din1 = sbc.tile([32, F], FP, tag="din1")
            nc.sync.dma_start(din1[:], eps_d.ap()[0:32, :])

            # ---------- scalar HWDGE: pk10a/b (outputs come later) ------------
            pk10a = sbc.tile([10, 136], FP, tag="pk10a")
            nc.scalar.dma_start(pk10a[:], pk10a_d.ap())
            pk10b = sbc.tile([10, 2304], FP, tag="pk10b")
            nc.scalar.dma_start(pk10b[:], pk10b_d.ap())

            # pk10a: [:,0:10]=P^T [:,10:20]=P [:,20:22]=Wm^T [:,22]=D [:,23]=b2
            #        [:,24:34]=I10 [:,34:134]=oht [0:8,134]=bm8
            pT = pk10a[:, 0:10]
            p_nat = pk10a[:, 10:20]
            wmT = pk10a[:, 20:22]
            dcol = pk10a[:, 22:23]
            b2col = pk10a[:, 23:24]
            idm = pk10a[:, 24:34]
            oht = pk10a[:, 34:134]
            bm8 = pk10a[0:8, 134:135]
            # pk10b: [:,0:256]=W2, [:,256:2304]=kf (0..2047 each row)
            w2 = pk10b[:, 0:256]
            kf = pk10b[:, 256:2304]
            # pk3: rows0-1=W1^T, row2=b1; col256=[t0,t1,1]
            tgt1 = pk3[:, 256:257]

            # ---------- lam = 1 - 0.01*exp(D); vc = lam^k ---------------------
            es = sbc.tile([H, 1], FP, tag="es")
            nc.scalar.activation(es[:], dcol, AFT.Exp)
            lam = sbc.tile([H, 1], FP, tag="lam")
            nc.vector.tensor_scalar(lam[:], es[:], -0.01, 1.0, ALU.mult, ALU.add)
            lnl = sbc.tile([H, 1], FP, tag="lnl")
            nc.scalar.activation(lnl[:], lam[:], AFT.Ln)
            vc = sbc.tile([H, N_STEPS], FP, tag="vc")
            nc.scalar.activation(vc[:], kf, AFT.Exp, scale=lnl[:])

            # ---------- h = relu(W1 t + b1) via augmented-contraction mm ------
            hp0 = psa.tile([128, 1], FP, tag="mm")
            nc.tensor.matmul(hp0[:], pk3[:, 0:128], tgt1)
            hp1 = psa.tile([128, 1], FP, tag="mm")
            nc.tensor.matmul(hp1[:], pk3[:, 128:256], tgt1)
            h0 = sbc.tile([128, 1], FP, tag="h0")
            nc.scalar.activation(h0[:], hp0[:], AFT.Relu)
            h1 = sbc.tile([128, 1], FP, tag="h1")
            nc.scalar.activation(h1[:], hp1[:], AFT.Relu)

            # ---------- Gauss-Jordan on [P^T | I] -> Q = P^-T -----------------
            aug = sbgj.tile([H, 2 * H], FP, tag="aug")
            nc.vector.tensor_copy(aug[:, 0:H], pT)
            nc.vector.tensor_copy(aug[:, H : 2 * H], idm)
            for k in range(H):
                fn = sbgj.tile([H, 1], FP, tag="fn")
                nc.gpsimd.tensor_sub(fn[:], idm[:, k : k + 1], aug[:, k : k + 1])
                bc = psbc.tile([H, 2 * H], FP, tag="bc")
                nc.tensor.matmul(bc[:], oht[:, H * k : H * k + H], aug[:])
                piv = sbgj.tile([H, 1], FP, tag="piv")
                nc.vector.reciprocal(piv[:], bc[:, k : k + 1])
                S = sbgj.tile([H, 2 * H], FP, tag="S")
                nc.vector.tensor_scalar_mul(S[:], bc[:], piv[:])
                aug2 = sbgj.tile([H, 2 * H], FP, tag="aug")
                nc.vector.scalar_tensor_tensor(
                    aug2[:], S[:], fn[:], aug[:], ALU.mult, ALU.add
                )
                aug = aug2

            # ---------- W2^T, x0, c, G^T*c ------------------------------------
            w2tp0 = psa.tile([128, H], FP, tag="mm")
            nc.tensor.matmul(w2tp0[:], w2[:, 0:128], idm, is_transpose=True)
            w2tp1 = psa.tile([128, H], FP, tag="mm")
            nc.tensor.matmul(w2tp1[:], w2[:, 128:256], idm, is_transpose=True)
            w2t0 = sbc.tile([128, H], FP, tag="w2t0")
            nc.vector.tensor_copy(w2t0[:], w2tp0[:])
            w2t1 = sbc.tile([128, H], FP, tag="w2t1")
            nc.vector.tensor_copy(w2t1[:], w2tp1[:])
            x0p = psa.tile([H, 1], FP, tag="mm")
            nc.tensor.matmul(x0p[:], w2t0[:], h0[:], start=True, stop=False)
            nc.tensor.matmul(x0p[:], w2t1[:], h1[:], start=False, stop=True)
            x0s = sbc.tile([H, 1], FP, tag="x0s")
            nc.scalar.activation(x0s[:], x0p[:], AFT.Identity, bias=b2col, scale=1.0)
            gtp = psa.tile([H, 2], FP, tag="mm")
            nc.tensor.matmul(gtp[:], p_nat, wmT)
            cp = psa.tile([H, 1], FP, tag="mm")
            nc.tensor.matmul(cp[:], aug[:, H : 2 * H], x0s[:])
            gts = sbc.tile([H, 2], FP, tag="gts")
            nc.vector.tensor_scalar_mul(gts[:], gtp[:], cp[:, 0:1])

            # ---------- actions: ONE [8,512] mm via lam^(512j) folding --------
            # gstack[:, 2j+ch] = gts[:, ch] * lam^(512j); vc cols 0,512,1024,1536
            gstack = sbc.tile([H, 8], FP, tag="gstack")
            for j in range(4):
                nc.vector.tensor_scalar_mul(
                    gstack[:, 2 * j : 2 * j + 2], gts[:], vc[:, 512 * j : 512 * j + 1]
                )
            pre8 = psa.tile([8, 512], FP, tag="mm")
            nc.tensor.matmul(pre8[:], gstack[:], vc[:, 0:512])
            ats8 = sbc.tile([8, 512], BF, tag="ats8")
            nc.scalar.activation(ats8[:], pre8[:], AFT.Tanh, bias=bm8, scale=1.0)

            # ---------- B[p, 2t+ch] = 15000*row_(2j+ch)(ats8) broadcast -------
            # copies split scalar (ch0) / vector (ch1) so B's halves complete
            # as the bcast matmuls stream out of the PE
            Bsb = sbc.tile([128, F], FP, tag="B")
            B3 = Bsb[:].rearrange("p (t m) -> p t m", m=2)
            for r in range(8):
                j, ch = r // 2, r % 2
                bp = psB.tile([128, 512], FP, tag="B")
                nc.tensor.matmul(bp[:], selm[:, 128 * r : 128 * (r + 1)], ats8[:])
                dst = B3[:, 512 * j : 512 * (j + 1), ch : ch + 1]
                if ch == 0:
                    nc.scalar.activation(dst, bp[:, :, None], AFT.Copy, scale=15000.0)
                else:
                    nc.vector.tensor_scalar(
                        dst, bp[:, :, None], 15000.0, None, ALU.mult
                    )

            # ---------- main: out = 150*eps + B (half-col STTs so work can ---
            # start on B's first half); one output DMA per tile, last tile
            # split + dummy so the tail is a tight co-finishing burst
            HW = F // 2
            for t, r0, pt in eps_tiles:
                for hh in range(2):
                    c0, c1 = hh * HW, (hh + 1) * HW
                    nc.vector.scalar_tensor_tensor(
                        t[0:pt, c0:c1],
                        t[0:pt, c0:c1],
                        150.0,
                        Bsb[0:pt, c0:c1],
                        ALU.mult,
                        ALU.add,
                    )
                if pt == 128:
                    nc.scalar.dma_start(out_d.ap()[r0 : r0 + pt, :], t[0:pt, :])
                else:
                    nc.scalar.dma_start(out_d.ap()[r0 : r0 + 57, :], t[0:57, :])
                    nc.scalar.dma_start(out_d.ap()[r0 + 57 : r0 + pt, :], t[57:pt, :])
                    nc.scalar.dma_start(dscr_d.ap(), Bsb[0:32, :])

    nc.compile()
    return nc


def get_nc():
    if "nc" not in _NC_CACHE:
        _NC_CACHE["nc"] = build_nc()
    return _NC_CACHE["nc"]


def _pack_smalls(inputs):
    f32 = lambda k: np.asarray(inputs[k], dtype=np.float32)
    pk3 = np.zeros((3, 257), dtype=np.float32)
    pk3[0:2, 0:256] = f32("W1").T
    pk3[2, 0:256] = f32("b1")
    pk3[0:2, 256] = f32("target")
    pk3[2, 256] = 1.0
    pk10a = np.zeros((10, 136), dtype=np.float32)
    pk10a[:, 0:10] = f32("P").T
    pk10a[:, 10:20] = f32("P")
    pk10a[:, 20:22] = f32("Wm").T
    pk10a[:, 22] = f32("D")
    pk10a[:, 23] = f32("b2")
    pk10a[:, 24:34] = np.eye(10, dtype=np.float32)
    pk10a[:, 34:134] = np.repeat(np.eye(10, dtype=np.float32), 10, axis=1)
    pk10a[0:8, 134] = np.tile(f32("bm"), 4)
    pk10b = np.zeros((10, 2304), dtype=np.float32)
    pk10b[:, 0:256] = f32("W2")
    pk10b[:, 256:2304] = np.arange(N_STEPS, dtype=np.float32)[None, :]
    return (
        np.ascontiguousarray(pk3),
        np.ascontiguousarray(pk10a),
        np.ascontiguousarray(pk10b),
    )


def kernel(**inputs):
    nc = get_nc()
    eps = np.ascontiguousarray(
        np.asarray(inputs["eps"], dtype=np.float32).reshape(N_ARMS, F)
    )
    pk3, pk10a, pk10b = _pack_smalls(inputs)
    in_maps = [
        {
            "pk3": pk3,
            "pk10a": pk10a,
            "pk10b": pk10b,
            "eps": eps[i * ARMS_PER_CORE : (i + 1) * ARMS_PER_CORE],
        }
        for i in range(N_CORES)
    ]
    res = run_bass_kernel_spmd(nc, in_maps, core_ids=list(range(N_CORES)))
    out = np.concatenate([res.results[i]["out"] for i in range(N_CORES)], axis=0)
    return out.reshape(N_ARMS, 2, N_STEPS)


# revision 28
# speedup vs baseline: 1.1015x; 1.1015x over previous
"""Trainium2 Bass kernel for the arm-sampling rollout problem.

Math: the reference's 2048-step scan x <- x - (A@x)*dt with
A = P diag(exp(D)) P^-1 has the closed form
    hidden[k] = P diag(lam_i^k) P^-1 x0,   lam_i = 1 - dt*exp(D_i)
so actions^T[ch, k] = tanh(sum_i G[ch,i] * c_i * lam_i^k + bm[ch]) with
G = Wm @ P and c = P^-1 x0 (on-device Gauss-Jordan on [P^T | I]).
The output is the memory-bound broadcast
    out[arm, j] = 150*eps[arm, j] + 15000*act_flat[j]
over a [5000, 4096] array, 625 arms per core across 8 cores.

Key scheduling facts measured from NTFF profiles:
- A dma_start's descriptors (16KB per partition-row) are spread over
  the 16 DMA engines (26.8GB/s each) only while other transfers are
  co-resident in the queue's dispatch window; a transfer alone in the
  window crawls on ~1 engine. Each HWDGE queue also only admits ~5
  outstanding dma_starts (later triggers stall). So bulk I/O is split
  into ~1MB transfers across BOTH HWDGE queues (sync+scalar), with
  small trailing dummy transfers so no real transfer drains alone.
- PE fp32 matmul costs ~2.1us per 512 cols; bf16 ~0.55us. The action
  row is computed as ONE [8,512] fp32 matmul via the geometric-series
  split lam^(k+512j) = lam^(512j) * lam^k (lam^(512j) columns folded
  into 8 stacked lhsT columns), tanh'd to bf16, and broadcast 128-wide
  by selector-matrix bf16 matmuls (rhs = full [8,512] tile at base
  partition 0, so no cross-partition hop is needed).
- All small parameters, the identity/row-selector tables, and the
  step-index row are host-packed into three small partition-layout
  arrays (pk3, pk10a, pk10b), each loading as a handful of
  descriptors; gpsimd runs nothing but the tiny GJ column preps.
"""

import numpy as np

import concourse.bass as bass
import concourse.bacc as bacc
import concourse.mybir as mybir
import concourse.tile as tile
from concourse.bass_utils import run_bass_kernel_spmd

N_ARMS = 5000
N_STEPS = 2048
H = 10
F = 2 * N_STEPS  # 4096 flattened per-arm elements
N_CORES = 8
ARMS_PER_CORE = N_ARMS // N_CORES  # 625
FP = mybir.dt.float32
BF = mybir.dt.bfloat16

_NC_CACHE: dict = {}


def build_nc():
    AFT = mybir.ActivationFunctionType
    ALU = mybir.AluOpType

    nc = bacc.Bacc(
        "TRN2",
        target_bir_lowering=False,
        debug=False,
        enable_asserts=True,
        num_devices=N_CORES,
    )

    eps_d = nc.dram_tensor("eps", [ARMS_PER_CORE, F], FP, kind="ExternalInput")
    pk3_d = nc.dram_tensor("pk3", [3, 257], FP, kind="ExternalInput")
    pk10a_d = nc.dram_tensor("pk10a", [10, 136], FP, kind="ExternalInput")
    pk10b_d = nc.dram_tensor("pk10b", [10, 2304], FP, kind="ExternalInput")
    out_d = nc.dram_tensor("out", [ARMS_PER_CORE, F], FP, kind="ExternalOutput")
    dscr_d = nc.dram_tensor("dscr", [32, F], FP, kind="Internal")

    # DMA facts (measured): descriptors are one 16KB row each and only 16KB
    # descriptors get spread across the 16 DMA engines (bigger ones crawl on
    # one engine); each queue's descriptor ring holds ~512 and admits whole
    # transfers only; transfers triggered back-to-back co-finish at ~430GB/s
    # aggregate, stragglers crawl. So: 4 full 128-row tiles burst on sync
    # (512 descriptors exactly) and the 113-row tile rides the scalar
    # queue's initial burst alongside the packed smalls.
    TILE_ROWS = [(0, 128), (128, 256), (256, 384), (384, 512), (512, 625)]

    with tile.TileContext(nc) as tc:
        with (
            tc.tile_pool(name="sbc", bufs=1) as sbc,
            tc.tile_pool(name="sbgj", bufs=2) as sbgj,
            tc.tile_pool(name="sbeps", bufs=1) as sbeps,
            tc.tile_pool(name="psa", bufs=3, space=bass.MemorySpace.PSUM) as psa,
            tc.tile_pool(name="psbc", bufs=2, space=bass.MemorySpace.PSUM) as psbc,
            tc.tile_pool(name="psB", bufs=2, space=bass.MemorySpace.PSUM) as psB,
        ):
            # ---------- selector table built pre-input ------------------------
            # selm[:, 128r:128(r+1)] is the [8,128] lhsT that broadcasts row r
            seli = sbc.tile([8, 8 * 128], mybir.dt.int32, tag="seli")
            nc.gpsimd.iota(
                seli[:], pattern=[[-1, 8], [0, 128]], base=0, channel_multiplier=1
            )
            selm = sbc.tile([8, 8 * 128], BF, tag="selm")
            nc.vector.tensor_scalar(selm[:], seli[:], 0, None, ALU.is_equal)

            # ---------- sync HWDGE: pk3 + input groups + dummy ----------------
            # Order per the measured credit/dealing behavior: small tile-4
            # pieces ride the first tight window; t2/t3 join as credits free.
            pk3 = sbc.tile([3, 257], FP, tag="pk3")
            nc.sync.dma_start(pk3[:], pk3_d.ap())
            eps_tiles = []
            for r0, r1 in TILE_ROWS:
                t = sbeps.tile([128, F], FP, tag="eps" + str(r0))
                eps_tiles.append((t, r0, r1 - r0))
            # {pk3,t0,t1,t4a,t4b,t2} = 500 descriptors, admitted as one
            # back-to-back burst (ring holds 512; slots free per completed
            # transfer); t3 is the single unavoidable late joiner
            IN_GROUPS = [
                (0, 128), (128, 256), (512, 569), (569, 625),
                (256, 384), (384, 512),
            ]
            for g0, g1 in IN_GROUPS:
                ti = min(g0 // 128, 4)
                t, r0, _ = eps_tiles[ti]
                nc.sync.dma_start(t[g0 - r0 : g1 - r0, :], eps_d.ap()[g0:g1, :])
            din1 = sbc.tile([32, F], FP, tag="din1")
            nc.sync.dma_start(din1[:], eps_d.ap()[0:32, :])

            # ---------- scalar HWDGE: pk10a/b (outputs come later) ------------
            pk10a = sbc.tile([10, 136], FP, tag="pk10a")
            nc.scalar.dma_start(pk10a[:], pk10a_d.ap())
            pk10b = sbc.tile([10, 2304], FP, tag="pk10b")
            nc.scalar.dma_start(pk10b[:], pk10b_d.ap())

            # pk10a: [:,0:10]=P^T [:,10:20]=P [:,20:22]=Wm^T [:,22]=D [:,23]=b2
            #        [:,24:34]=I10 [:,34:134]=oht [0:8,134]=bm8
            pT = pk10a[:, 0:10]
            p_nat = pk10a[:, 10:20]
            wmT = pk10a[:, 20:22]
            dcol = pk10a[:, 22:23]
            b2col = pk10a[:, 23:24]
            idm = pk10a[:, 24:34]
            oht = pk10a[:, 34:134]
            bm8 = pk10a[0:8, 134:135]
            # pk10b: [:,0:256]=W2, [:,256:2304]=kf (0..2047 each row)
            w2 = pk10b[:, 0:256]
            kf = pk10b[:, 256:2304]
            # pk3: rows0-1=W1^T, row2=b1; col256=[t0,t1,1]
            tgt1 = pk3[:, 256:257]

            # ---------- lam = 1 - 0.01*exp(D); vc = lam^k ---------------------
            es = sbc.tile([H, 1], FP, tag="es")
            nc.scalar.activation(es[:], dcol, AFT.Exp)
            lam = sbc.tile([H, 1], FP, tag="lam")
            nc.vector.tensor_scalar(lam[:], es[:], -0.01, 1.0, ALU.mult, ALU.add)
            lnl = sbc.tile([H, 1], FP, tag="lnl")
            nc.scalar.activation(lnl[:], lam[:], AFT.Ln)
            vc = sbc.tile([H, N_STEPS], FP, tag="vc")
            nc.scalar.activation(vc[:], kf, AFT.Exp, scale=lnl[:])

            # ---------- h = relu(W1 t + b1) via augmented-contraction mm ------
            hp0 = psa.tile([128, 1], FP, tag="mm")
            nc.tensor.matmul(hp0[:], pk3[:, 0:128], tgt1)
            hp1 = psa.tile([128, 1], FP, tag="mm")
            nc.tensor.matmul(hp1[:], pk3[:, 128:256], tgt1)
            h0 = sbc.tile([128, 1], FP, tag="h0")
            nc.scalar.activation(h0[:], hp0[:], AFT.Relu)
            h1 = sbc.tile([128, 1], FP, tag="h1")
            nc.scalar.activation(h1[:], hp1[:], AFT.Relu)

            # ---------- Gauss-Jordan on [P^T | I] -> Q = P^-T -----------------
            aug = sbgj.tile([H, 2 * H], FP, tag="aug")
            nc.vector.tensor_copy(aug[:, 0:H], pT)
            nc.vector.tensor_copy(aug[:, H : 2 * H], idm)
            for k in range(H):
                fn = sbgj.tile([H, 1], FP, tag="fn")
                nc.gpsimd.tensor_sub(fn[:], idm[:, k : k + 1], aug[:, k : k + 1])
                bc = psbc.tile([H, 2 * H], FP, tag="bc")
                nc.tensor.matmul(bc[:], oht[:, H * k : H * k + H], aug[:])
                piv = sbgj.tile([H, 1], FP, tag="piv")
                nc.vector.reciprocal(piv[:], bc[:, k : k + 1])
                S = sbgj.tile([H, 2 * H], FP, tag="S")
                nc.vector.tensor_scalar_mul(S[:], bc[:], piv[:])
                aug2 = sbgj.tile([H, 2 * H], FP, tag="aug")
                nc.vector.scalar_tensor_tensor(
                    aug2[:], S[:], fn[:], aug[:], ALU.mult, ALU.add
                )
                aug = aug2

            # ---------- W2^T, x0, c, G^T*c ------------------------------------
            w2tp0 = psa.tile([128, H], FP, tag="mm")
            nc.tensor.matmul(w2tp0[:], w2[:, 0:128], idm, is_transpose=True)
            w2tp1 = psa.tile([128, H], FP, tag="mm")
            nc.tensor.matmul(w2tp1[:], w2[:, 128:256], idm, is_transpose=True)
            w2t0 = sbc.tile([128, H], FP, tag="w2t0")
            nc.vector.tensor_copy(w2t0[:], w2tp0[:])
            w2t1 = sbc.tile([128, H], FP, tag="w2t1")
            nc.vector.tensor_copy(w2t1[:], w2tp1[:])
            x0p = psa.tile([H, 1], FP, tag="mm")
            nc.tensor.matmul(x0p[:], w2t0[:], h0[:], start=True, stop=False)
            nc.tensor.matmul(x0p[:], w2t1[:], h1[:], start=False, stop=True)
            x0s = sbc.tile([H, 1], FP, tag="x0s")
            nc.scalar.activation(x0s[:], x0p[:], AFT.Identity, bias=b2col, scale=1.0)
            gtp = psa.tile([H, 2], FP, tag="mm")
            nc.tensor.matmul(gtp[:], p_nat, wmT)
            cp = psa.tile([H, 1], FP, tag="mm")
            nc.tensor.matmul(cp[:], aug[:, H : 2 * H], x0s[:])
            gts = sbc.tile([H, 2], FP, tag="gts")
            nc.vector.tensor_scalar_mul(gts[:], gtp[:], cp[:, 0:1])

            # ---------- actions: ONE [8,512] mm via lam^(512j) folding --------
            # gstack[:, 2j+ch] = gts[:, ch] * lam^(512j); vc cols 0,512,1024,1536
            gstack = sbc.tile([H, 8], FP, tag="gstack")
            for j in range(4):
                nc.vector.tensor_scalar_mul(
                    gstack[:, 2 * j : 2 * j + 2], gts[:], vc[:, 512 * j : 512 * j + 1]
                )
            pre8 = psa.tile([8, 512], FP, tag="mm")
            nc.tensor.matmul(pre8[:], gstack[:], vc[:, 0:512])
            ats8 = sbc.tile([8, 512], BF, tag="ats8")
            nc.scalar.activation(ats8[:], pre8[:], AFT.Tanh, bias=bm8, scale=1.0)

            # ---------- B[p, 2t+ch] = 15000*row_(2j+ch)(ats8) broadcast -------
            # copies split scalar (ch0) / vector (ch1) so B's halves complete
            # as the bcast matmuls stream out of the PE
            Bsb = sbc.tile([128, F], FP, tag="B")
            B3 = Bsb[:].rearrange("p (t m) -> p t m", m=2)
            for r in range(8):
                j, ch = r // 2, r % 2
                bp = psB.tile([128, 512], FP, tag="B")
                nc.tensor.matmul(bp[:], selm[:, 128 * r : 128 * (r + 1)], ats8[:])
                dst = B3[:, 512 * j : 512 * (j + 1), ch : ch + 1]
                if ch == 0:
                    nc.scalar.activation(dst, bp[:, :, None], AFT.Copy, scale=15000.0)
                else:
                    nc.vector.tensor_scalar(
                        dst, bp[:, :, None], 15000.0, None, ALU.mult
                    )

            # ---------- main: out = 150*eps + B (half-col STTs so work can ---
            # start on B's first half); one output DMA per tile, last tile
            # split + dummy so the tail is a tight co-finishing burst
            HW = F // 2
            for t, r0, pt in eps_tiles:
                for hh in range(2):
                    c0, c1 = hh * HW, (hh + 1) * HW
                    nc.vector.scalar_tensor_tensor(
                        t[0:pt, c0:c1],
                        t[0:pt, c0:c1],
                        150.0,
                        Bsb[0:pt, c0:c1],
                        ALU.mult,
                        ALU.add,
                    )
                if pt == 128:
                    nc.scalar.dma_start(out_d.ap()[r0 : r0 + pt, :], t[0:pt, :])
                else:
                    nc.scalar.dma_start(out_d.ap()[r0 : r0 + 57, :], t[0:57, :])
                    nc.scalar.dma_start(out_d.ap()[r0 + 57 : r0 + pt, :], t[57:pt, :])
                    nc.scalar.dma_start(dscr_d.ap(), Bsb[0:32, :])

    nc.compile()
    return nc


def get_nc():
    if "nc" not in _NC_CACHE:
        _NC_CACHE["nc"] = build_nc()
    return _NC_CACHE["nc"]


def _pack_smalls(inputs):
    f32 = lambda k: np.asarray(inputs[k], dtype=np.float32)
    pk3 = np.zeros((3, 257), dtype=np.float32)
    pk3[0:2, 0:256] = f32("W1").T
    pk3[2, 0:256] = f32("b1")
    pk3[0:2, 256] = f32("target")
    pk3[2, 256] = 1.0
    pk10a = np.zeros((10, 136), dtype=np.float32)
    pk10a[:, 0:10] = f32("P").T
    pk10a[:, 10:20] = f32("P")
    pk10a[:, 20:22] = f32("Wm").T
    pk10a[:, 22] = f32("D")
    pk10a[:, 23] = f32("b2")
    pk10a[:, 24:34] = np.eye(10, dtype=np.float32)
    pk10a[:, 34:134] = np.repeat(np.eye(10, dtype=np.float32), 10, axis=1)
    pk10a[0:8, 134] = np.tile(f32("bm"), 4)
    pk10b = np.zeros((10, 2304), dtype=np.float32)
    pk10b[:, 0:256] = f32("W2")
    pk10b[:, 256:2304] = np.arange(N_STEPS, dtype=np.float32)[None, :]
    return (
        np.ascontiguousarray(pk3),
        np.ascontiguousarray(pk10a),
        np.ascontiguousarray(pk10b),
    )


def kernel(**inputs):
    nc = get_nc()
    eps = np.ascontiguousarray(
        np.asarray(inputs["eps"], dtype=np.float32).reshape(N_ARMS, F)
    )
    pk3, pk10a, pk10b = _pack_smalls(inputs)
    in_maps = [
        {
            "pk3": pk3,
            "pk10a": pk10a,
            "pk10b": pk10b,
            "eps": eps[i * ARMS_PER_CORE : (i + 1) * ARMS_PER_CORE],
        }
        for i in range(N_CORES)
    ]
    res = run_bass_kernel_spmd(nc, in_maps, core_ids=list(range(N_CORES)))
    out = np.concatenate([res.results[i]["out"] for i in range(N_CORES)], axis=0)
    return out.reshape(N_ARMS, 2, N_STEPS)
